# revision 44
# baseline (speedup 1.0000x reference)
"""Trainium2 Bass kernel for nn_DualSPRTLinear: out = x @ (ternary*scales).T

Shapes (hardcoded):
  x       [4, 2048, 4096] fp32   -> tokens T=8192, in-features K=4096
  ternary [4096, 4096]    int8   (out-features O x K), values in {-1,0,1}
  scales  [131072]        fp32   one positive scale per contiguous group of
                                 128 weights (row-major over [O, K]), i.e.
                                 w[o,k] = ternary[o,k] * scales[o*32 + k//128]
  out     [4, 2048, 4096] fp32

Strategy: data-parallel over tokens across 8 NeuronCores (1024 tokens/core;
~34 GFLOP/core, BF16 PE roofline ~437 us). Each core holds its x-slice
transposed ([K, 1024] bf16, SBUF-resident) and streams the full ternary
weight (int8, transposed to [K, O]) plus pre-broadcast group scales (bf16)
from HBM, dequantizing on VectorE (mixed int8 x bf16 -> bf16 multiply) and
matmul'ing on TensorE (lhsT = x tile [128k x 128t] stationary, rhs = w tile
[128k x 512o] streaming, fp32 PSUM accumulation over the 32 k-chunks, which
exactly match the quantization groups). Host work is layout-only: transpose,
dtype casts, scale replication, shard/concat.
"""

import os
import sys

import numpy as np

for _p in ("/opt/trn_rl_repo",):
    if _p not in sys.path and os.path.isdir(_p):
        sys.path.append(_p)

import ml_dtypes

import concourse.bacc as bacc
import concourse.mybir as mybir
import concourse.tile as tile
from concourse.bass_utils import run_bass_kernel_spmd

BF16 = ml_dtypes.bfloat16

_AXON_SO = "/opt/axon/libaxon_pjrt.so"


def _ensure_ntff_hook():
    """The agent image's ``antenv`` lacks ``axon_hooks``, so the boot shim
    skips NTFF-hook registration and ``run_bass_kernel_spmd(trace=True)``
    crashes on import. Recreate the module + hook via ctypes on the axon
    PJRT .so (same ABI the boot script uses)."""
    import types

    if "antenv.axon_hooks" in sys.modules:
        return
    import contextlib
    import ctypes

    import antenv

    mod = types.ModuleType("antenv.axon_hooks")
    _state = {"hook": None}
    mod.set_axon_ntff_profile_hook = lambda h: _state.__setitem__("hook", h)
    mod.get_axon_ntff_profile_hook = lambda: _state["hook"]
    sys.modules["antenv.axon_hooks"] = mod
    antenv.axon_hooks = mod

    if not os.path.exists(_AXON_SO):
        return
    lib = ctypes.CDLL(_AXON_SO)
    if not hasattr(lib, "axon_start_nrt_profile"):
        return
    lib.axon_start_nrt_profile.argtypes = [
        ctypes.POINTER(ctypes.c_int64),
        ctypes.c_size_t,
    ]
    lib.axon_start_nrt_profile.restype = ctypes.c_int64
    lib.axon_stop_nrt_profile.argtypes = [ctypes.c_char_p]
    lib.axon_stop_nrt_profile.restype = ctypes.c_int64

    @contextlib.contextmanager
    def _hook(output_dir, device_ids):
        import jax

        jax.devices()
        if device_ids:
            ids = (ctypes.c_int64 * len(device_ids))(*device_ids)
            rc = lib.axon_start_nrt_profile(ids, len(device_ids))
        else:
            rc = lib.axon_start_nrt_profile(None, 0)
        if rc != 0:
            raise RuntimeError(f"axon_start_nrt_profile rc={rc}")
        try:
            yield
        finally:
            n = lib.axon_stop_nrt_profile(str(output_dir).encode())
            print(f"profile: {n} file(s) written to {output_dir}", file=sys.stderr)

    _state["hook"] = _hook

N_CORES = 8
T = 8192          # total tokens
TC = T // N_CORES # tokens per core = 1024
K = 4096          # in-features (contraction)
O = 4096          # out-features
GS = 128          # scale group size == matmul k-chunk
NG = K // GS      # 32 k-chunks
OB = 512          # o-block (matmul free dim / one PSUM bank of fp32)
NJ = O // OB      # 8 o-blocks
GG = 8            # k-chunks per DMA super-tile
NGG = NG // GG    # 4 super-tiles
NM = TC // 128    # 8 token blocks per core


def _build():
    nc = bacc.Bacc(None, target_bir_lowering=False, debug=False)
    xt = nc.dram_tensor("xt", [K, TC], mybir.dt.bfloat16, kind="ExternalInput")
    tt = nc.dram_tensor("tt", [K, O], mybir.dt.int8, kind="ExternalInput")
    sb = nc.dram_tensor("sb", [NG, 128, O], mybir.dt.bfloat16, kind="ExternalInput")
    out = nc.dram_tensor("out", [TC, O], mybir.dt.float32, kind="ExternalOutput")

    xt_r = xt[:].rearrange("(g p) t -> p g t", p=128)   # [128, 32, 1024]
    tt_r = tt[:].rearrange("(g p) o -> p g o", p=128)   # [128, 32, 4096]
    sb_a = sb[:]                                        # [32, 128, 4096]
    out_a = out[:]                                      # [1024, 4096]

    with tile.TileContext(nc) as tc:
        with (
            tc.tile_pool(name="xres", bufs=NG // 4) as xpool,
            tc.tile_pool(name="tern", bufs=4) as tpool,
            tc.tile_pool(name="scal", bufs=3) as spool,
            tc.tile_pool(name="wdeq", bufs=6) as wpool,
            tc.tile_pool(name="ostg", bufs=4) as opool,
            tc.tile_pool(name="warm", bufs=1) as warmpool,
            tc.tile_pool(name="psum", bufs=8, space="PSUM") as ppool,
        ):
            # x slice, transposed+bf16 on host, resident in SBUF for the
            # whole kernel: 8 batches of 4 k-chunks [128, 4, 1024] = 8 MiB,
            # interleaved with j0's weight stream on the sync ring so each
            # batch lands just before its matmuls need it.
            x_bat = [None] * (NG // 4)

            def load_x_batch(b, split_first=False, eng=None):
                eng = eng or nc.sync
                xb = xpool.tile(
                    [128, 4, TC], mybir.dt.bfloat16, name=f"x_{b}", tag="xg"
                )
                if split_first:
                    # chunk g=0 alone: matmul #1 waits only on 256 KiB
                    eng.dma_start(xb[:, :1, :], xt_r[:, :1, :])
                    eng.dma_start(xb[:, 1:, :], xt_r[:, 1:4, :])
                else:
                    eng.dma_start(xb[:], xt_r[:, 4 * b : 4 * b + 4, :])
                x_bat[b] = xb

            # PE warm-up: throwaway matmuls bridging the preamble + first
            # weight tile's DMA/dequant latency, so the HAM clock gate is at
            # 2.4 GHz and the PE never idles (= never re-throttles) when the
            # real stream starts. Writes land in a psum slot that is
            # released before the real tiles need it.
            warm_sb = warmpool.tile([128, OB], mybir.dt.bfloat16)
            nc.vector.memset(warm_sb[:], 0.0)
            warm_ps = ppool.tile([128, OB], mybir.dt.float32, name="ps_warm", tag="ps")
            for _ in range(38):
                nc.tensor.matmul(
                    warm_ps[:], warm_sb[:, :128], warm_sb[:], start=True, stop=True
                )

            for j in range(NJ):  # output-feature blocks of 512
                osl = slice(j * OB, (j + 1) * OB)
                psum_tiles = [
                    ppool.tile(
                        [128, OB], mybir.dt.float32, name=f"ps_{j}_{m}", tag="ps"
                    )
                    for m in range(NM)
                ]
                # k super-tiles; j0's first ones are small so the first
                # real matmul is ready as early as possible
                widths = (4, 4, 8, 8, 8) if j == 0 else (GG,) * NGG
                xsched = {0: (0,), 1: (1,), 2: (2, 3), 3: (4, 5), 4: (6, 7)}
                g0 = 0
                def emit_mms(w_tile, g0, width, m_range):
                    for q in range(width):
                        g = g0 + q
                        for m in m_range:
                            nc.tensor.matmul(
                                psum_tiles[m][:],
                                x_bat[g // 4][:, g % 4, m * 128 : (m + 1) * 128],
                                w_tile[:, q, :],
                                start=(g == 0),
                                stop=(g == NG - 1),
                            )

                def evict(m, last_j):
                    o_tile = opool.tile(
                        [128, OB], mybir.dt.float32, name=f"o_{j}_{m}", tag="o"
                    )
                    # evictions on VectorE (cheap vs its dequant load now
                    # that halves overlap them with the other half's MMs)
                    nc.vector.tensor_copy(o_tile[:], psum_tiles[m][:])
                    nc.scalar.dma_start(
                        out_a[m * 128 : (m + 1) * 128, osl], o_tile[:]
                    )

                w_tiles = []
                for st, width in enumerate(widths):
                    gsl = slice(g0, g0 + width)
                    weng = nc.scalar if (j == 0 and st < 2) else nc.sync
                    t_tile = tpool.tile(
                        [128, width, OB], mybir.dt.int8,
                        name=f"t_{j}_{st}", tag="t",
                    )
                    weng.dma_start(t_tile[:], tt_r[:, gsl, osl])
                    s_tile = spool.tile(
                        [128, width, OB], mybir.dt.bfloat16,
                        name=f"s_{j}_{st}", tag="s",
                    )
                    weng.dma_start(
                        s_tile[:], sb_a[gsl, :, osl].rearrange("g p o -> p g o")
                    )
                    w_tile = wpool.tile(
                        [128, width, OB], mybir.dt.bfloat16,
                        name=f"w_{j}_{st}", tag="w",
                    )
                    nc.vector.tensor_tensor(
                        w_tile[:], t_tile[:], s_tile[:], mybir.AluOpType.mult
                    )
                    if j == 0:
                        for b in xsched.get(st, ()):
                            load_x_batch(b, split_first=(b == 0))
                        # j0: full-m sweep per super-tile (x batches arrive
                        # at the pace of this sweep)
                        emit_mms(w_tile, g0, width, range(NM))
                    w_tiles.append((w_tile, g0, width))
                    g0 += width
                if j == 0:
                    for m in range(NM):
                        evict(m, False)
                else:
                    # token-halves: each half is a full k-sweep over the
                    # resident dequantized super-tiles, so one half's
                    # evictions overlap the other half's matmuls and j
                    # boundaries never stall on PSUM recycling
                    # last j: small final half so the kernel tail is only
                    # two evictions + one store
                    halves = (
                        (range(0, 6), range(6, NM))
                        if j == NJ - 1
                        else (range(0, NM // 2), range(NM // 2, NM))
                    )
                    for half in halves:
                        for w_tile, wg0, wwidth in w_tiles:
                            emit_mms(w_tile, wg0, wwidth, half)
                        for m in half:
                            evict(m, j == NJ - 1)

    nc.compile()
    return nc


_NC = None


def _get_nc():
    global _NC
    if _NC is None:
        _NC = _build()
    return _NC


def _prep_inputs(x, ternary, scales):
    x = np.asarray(x)
    ternary = np.asarray(ternary)
    scales = np.asarray(scales)

    xt = np.ascontiguousarray(x.reshape(T, K).astype(BF16).T)       # [K, T]
    tt = np.ascontiguousarray(ternary.astype(np.int8).T)            # [K, O]
    s_r = scales.reshape(O, NG).T.astype(BF16)                       # [NG, O]
    sb = np.ascontiguousarray(
        np.broadcast_to(s_r[:, None, :], (NG, 128, O))
    )  # [NG, 128, O]

    in_maps = []
    for c in range(N_CORES):
        in_maps.append(
            {
                "xt": np.ascontiguousarray(xt[:, c * TC : (c + 1) * TC]),
                "tt": tt,
                "sb": sb,
            }
        )
    return in_maps


def run(x, ternary, scales, trace=False, **trace_kwargs):
    """Run on 8 NeuronCores; returns (out [4,2048,4096] fp32, BassKernelResults)."""
    nc = _get_nc()
    if trace:
        _ensure_ntff_hook()
    in_maps = _prep_inputs(x, ternary, scales)
    res = run_bass_kernel_spmd(
        nc, in_maps, core_ids=list(range(N_CORES)), trace=trace, **trace_kwargs
    )
    parts = [np.asarray(r["out"]) for r in res.results]
    out = np.concatenate(parts, axis=0).reshape(4, 2048, O).astype(np.float32)
    return out, res


def kernel(x, ternary, scales):
    out, _ = run(x, ternary, scales, trace=False)
    return out


# revision 45
# speedup vs baseline: 1.0113x; 1.0113x over previous
"""Trainium2 Bass kernel for nn_DualSPRTLinear: out = x @ (ternary*scales).T

Shapes (hardcoded):
  x       [4, 2048, 4096] fp32   -> tokens T=8192, in-features K=4096
  ternary [4096, 4096]    int8   (out-features O x K), values in {-1,0,1}
  scales  [131072]        fp32   one positive scale per contiguous group of
                                 128 weights (row-major over [O, K]), i.e.
                                 w[o,k] = ternary[o,k] * scales[o*32 + k//128]
  out     [4, 2048, 4096] fp32

Strategy: data-parallel over tokens across 8 NeuronCores (1024 tokens/core;
~34 GFLOP/core, BF16 PE roofline ~437 us). Each core holds its x-slice
transposed ([K, 1024] bf16, SBUF-resident) and streams the full ternary
weight (int8, transposed to [K, O]) plus pre-broadcast group scales (bf16)
from HBM, dequantizing on VectorE (mixed int8 x bf16 -> bf16 multiply) and
matmul'ing on TensorE (lhsT = x tile [128k x 128t] stationary, rhs = w tile
[128k x 512o] streaming, fp32 PSUM accumulation over the 32 k-chunks, which
exactly match the quantization groups). Host work is layout-only: transpose,
dtype casts, scale replication, shard/concat.
"""

import os
import sys

import numpy as np

for _p in ("/opt/trn_rl_repo",):
    if _p not in sys.path and os.path.isdir(_p):
        sys.path.append(_p)

import ml_dtypes

import concourse.bacc as bacc
import concourse.mybir as mybir
import concourse.tile as tile
from concourse.bass_utils import run_bass_kernel_spmd

BF16 = ml_dtypes.bfloat16

_AXON_SO = "/opt/axon/libaxon_pjrt.so"


def _ensure_ntff_hook():
    """The agent image's ``antenv`` lacks ``axon_hooks``, so the boot shim
    skips NTFF-hook registration and ``run_bass_kernel_spmd(trace=True)``
    crashes on import. Recreate the module + hook via ctypes on the axon
    PJRT .so (same ABI the boot script uses)."""
    import types

    if "antenv.axon_hooks" in sys.modules:
        return
    import contextlib
    import ctypes

    import antenv

    mod = types.ModuleType("antenv.axon_hooks")
    _state = {"hook": None}
    mod.set_axon_ntff_profile_hook = lambda h: _state.__setitem__("hook", h)
    mod.get_axon_ntff_profile_hook = lambda: _state["hook"]
    sys.modules["antenv.axon_hooks"] = mod
    antenv.axon_hooks = mod

    if not os.path.exists(_AXON_SO):
        return
    lib = ctypes.CDLL(_AXON_SO)
    if not hasattr(lib, "axon_start_nrt_profile"):
        return
    lib.axon_start_nrt_profile.argtypes = [
        ctypes.POINTER(ctypes.c_int64),
        ctypes.c_size_t,
    ]
    lib.axon_start_nrt_profile.restype = ctypes.c_int64
    lib.axon_stop_nrt_profile.argtypes = [ctypes.c_char_p]
    lib.axon_stop_nrt_profile.restype = ctypes.c_int64

    @contextlib.contextmanager
    def _hook(output_dir, device_ids):
        import jax

        jax.devices()
        if device_ids:
            ids = (ctypes.c_int64 * len(device_ids))(*device_ids)
            rc = lib.axon_start_nrt_profile(ids, len(device_ids))
        else:
            rc = lib.axon_start_nrt_profile(None, 0)
        if rc != 0:
            raise RuntimeError(f"axon_start_nrt_profile rc={rc}")
        try:
            yield
        finally:
            n = lib.axon_stop_nrt_profile(str(output_dir).encode())
            print(f"profile: {n} file(s) written to {output_dir}", file=sys.stderr)

    _state["hook"] = _hook

N_CORES = 8
T = 8192          # total tokens
TC = T // N_CORES # tokens per core = 1024
K = 4096          # in-features (contraction)
O = 4096          # out-features
GS = 128          # scale group size == matmul k-chunk
NG = K // GS      # 32 k-chunks
OB = 512          # o-block (matmul free dim / one PSUM bank of fp32)
NJ = O // OB      # 8 o-blocks
GG = 8            # k-chunks per DMA super-tile
NGG = NG // GG    # 4 super-tiles
NM = TC // 128    # 8 token blocks per core


def _build():
    nc = bacc.Bacc(None, target_bir_lowering=False, debug=False)
    xt = nc.dram_tensor("xt", [K, TC], mybir.dt.bfloat16, kind="ExternalInput")
    tt = nc.dram_tensor("tt", [K, O], mybir.dt.int8, kind="ExternalInput")
    sb = nc.dram_tensor("sb", [NG, 128, O], mybir.dt.bfloat16, kind="ExternalInput")
    out = nc.dram_tensor("out", [TC, O], mybir.dt.float32, kind="ExternalOutput")

    xt_r = xt[:].rearrange("(g p) t -> p g t", p=128)   # [128, 32, 1024]
    tt_r = tt[:].rearrange("(g p) o -> p g o", p=128)   # [128, 32, 4096]
    sb_a = sb[:]                                        # [32, 128, 4096]
    out_a = out[:]                                      # [1024, 4096]

    with tile.TileContext(nc) as tc:
        with (
            tc.tile_pool(name="xres", bufs=NG // 4) as xpool,
            tc.tile_pool(name="tern", bufs=4) as tpool,
            tc.tile_pool(name="scal", bufs=3) as spool,
            tc.tile_pool(name="wdeq", bufs=6) as wpool,
            tc.tile_pool(name="ostg", bufs=4) as opool,
            tc.tile_pool(name="warm", bufs=1) as warmpool,
            tc.tile_pool(name="psum", bufs=8, space="PSUM") as ppool,
        ):
            # x slice, transposed+bf16 on host, resident in SBUF for the
            # whole kernel: 8 batches of 4 k-chunks [128, 4, 1024] = 8 MiB,
            # interleaved with j0's weight stream on the sync ring so each
            # batch lands just before its matmuls need it.
            x_bat = [None] * (NG // 4)

            def load_x_batch(b, split_first=False, eng=None):
                eng = eng or nc.sync
                xb = xpool.tile(
                    [128, 4, TC], mybir.dt.bfloat16, name=f"x_{b}", tag="xg"
                )
                if split_first:
                    # chunk g=0 alone: matmul #1 waits only on 256 KiB
                    eng.dma_start(xb[:, :1, :], xt_r[:, :1, :])
                    eng.dma_start(xb[:, 1:, :], xt_r[:, 1:4, :])
                else:
                    eng.dma_start(xb[:], xt_r[:, 4 * b : 4 * b + 4, :])
                x_bat[b] = xb

            # PE warm-up: throwaway matmuls bridging the preamble + first
            # weight tile's DMA/dequant latency, so the HAM clock gate is at
            # 2.4 GHz and the PE never idles (= never re-throttles) when the
            # real stream starts. Writes land in a psum slot that is
            # released before the real tiles need it.
            warm_sb = warmpool.tile([128, OB], mybir.dt.bfloat16)
            nc.vector.memset(warm_sb[:], 0.0)
            warm_ps = ppool.tile([128, OB], mybir.dt.float32, name="ps_warm", tag="ps")
            for _ in range(38):
                nc.tensor.matmul(
                    warm_ps[:], warm_sb[:, :128], warm_sb[:], start=True, stop=True
                )

            for j in range(NJ):  # output-feature blocks of 512
                osl = slice(j * OB, (j + 1) * OB)
                psum_tiles = [
                    ppool.tile(
                        [128, OB], mybir.dt.float32, name=f"ps_{j}_{m}", tag="ps"
                    )
                    for m in range(NM)
                ]
                # k super-tiles; j0's first ones are small so the first
                # real matmul is ready as early as possible
                widths = (4, 4, 8, 8, 8) if j == 0 else (GG,) * NGG
                xsched = {0: (0,), 1: (1,), 2: (2, 3), 3: (4, 5), 4: (6, 7)}
                g0 = 0
                def emit_mms(w_tile, g0, width, m_range):
                    for q in range(width):
                        g = g0 + q
                        for m in m_range:
                            nc.tensor.matmul(
                                psum_tiles[m][:],
                                x_bat[g // 4][:, g % 4, m * 128 : (m + 1) * 128],
                                w_tile[:, q, :],
                                start=(g == 0),
                                stop=(g == NG - 1),
                            )

                def evict(m, last_j):
                    o_tile = opool.tile(
                        [128, OB], mybir.dt.float32, name=f"o_{j}_{m}", tag="o"
                    )
                    # mid-kernel evictions live on ScalarE so VectorE is
                    # always free for the next dequant; the final j
                    # alternates engines to halve the kernel tail
                    if last_j and m % 2 == 1:
                        nc.vector.tensor_copy(o_tile[:], psum_tiles[m][:])
                    else:
                        nc.scalar.copy(o_tile[:], psum_tiles[m][:])
                    nc.scalar.dma_start(
                        out_a[m * 128 : (m + 1) * 128, osl], o_tile[:]
                    )

                w_tiles = []
                for st, width in enumerate(widths):
                    gsl = slice(g0, g0 + width)
                    weng = nc.scalar if (j == 0 and st < 2) else nc.sync
                    t_tile = tpool.tile(
                        [128, width, OB], mybir.dt.int8,
                        name=f"t_{j}_{st}", tag="t",
                    )
                    weng.dma_start(t_tile[:], tt_r[:, gsl, osl])
                    s_tile = spool.tile(
                        [128, width, OB], mybir.dt.bfloat16,
                        name=f"s_{j}_{st}", tag="s",
                    )
                    weng.dma_start(
                        s_tile[:], sb_a[gsl, :, osl].rearrange("g p o -> p g o")
                    )
                    w_tile = wpool.tile(
                        [128, width, OB], mybir.dt.bfloat16,
                        name=f"w_{j}_{st}", tag="w",
                    )
                    nc.vector.tensor_tensor(
                        w_tile[:], t_tile[:], s_tile[:], mybir.AluOpType.mult
                    )
                    if j == 0:
                        for b in xsched.get(st, ()):
                            load_x_batch(b, split_first=(b == 0))
                        # j0: full-m sweep per super-tile (x batches arrive
                        # at the pace of this sweep)
                        emit_mms(w_tile, g0, width, range(NM))
                    w_tiles.append((w_tile, g0, width))
                    g0 += width
                if j == 0:
                    for m in range(NM):
                        evict(m, False)
                else:
                    # token-halves: each half is a full k-sweep over the
                    # resident dequantized super-tiles, so one half's
                    # evictions overlap the other half's matmuls and j
                    # boundaries never stall on PSUM recycling
                    # last j: small final half so the kernel tail is only
                    # two evictions + one store
                    halves = (
                        (range(0, 6), range(6, NM))
                        if j == NJ - 1
                        else (range(0, NM // 2), range(NM // 2, NM))
                    )
                    for half in halves:
                        for w_tile, wg0, wwidth in w_tiles:
                            emit_mms(w_tile, wg0, wwidth, half)
                        for m in half:
                            evict(m, j == NJ - 1)

    nc.compile()
    return nc


_NC = None


def _get_nc():
    global _NC
    if _NC is None:
        _NC = _build()
    return _NC


def _prep_inputs(x, ternary, scales):
    x = np.asarray(x)
    ternary = np.asarray(ternary)
    scales = np.asarray(scales)

    xt = np.ascontiguousarray(x.reshape(T, K).astype(BF16).T)       # [K, T]
    tt = np.ascontiguousarray(ternary.astype(np.int8).T)            # [K, O]
    s_r = scales.reshape(O, NG).T.astype(BF16)                       # [NG, O]
    sb = np.ascontiguousarray(
        np.broadcast_to(s_r[:, None, :], (NG, 128, O))
    )  # [NG, 128, O]

    in_maps = []
    for c in range(N_CORES):
        in_maps.append(
            {
                "xt": np.ascontiguousarray(xt[:, c * TC : (c + 1) * TC]),
                "tt": tt,
                "sb": sb,
            }
        )
    return in_maps


def run(x, ternary, scales, trace=False, **trace_kwargs):
    """Run on 8 NeuronCores; returns (out [4,2048,4096] fp32, BassKernelResults)."""
    nc = _get_nc()
    if trace:
        _ensure_ntff_hook()
    in_maps = _prep_inputs(x, ternary, scales)
    res = run_bass_kernel_spmd(
        nc, in_maps, core_ids=list(range(N_CORES)), trace=trace, **trace_kwargs
    )
    parts = [np.asarray(r["out"]) for r in res.results]
    out = np.concatenate(parts, axis=0).reshape(4, 2048, O).astype(np.float32)
    return out, res


def kernel(x, ternary, scales):
    out, _ = run(x, ternary, scales, trace=False)
    return out
